# revision 26
# baseline (speedup 1.0000x reference)
"""GNN message-passing layer on 8 TRN2 NeuronCores.

Computes out = relu((adj^T @ x / deg) @ U^T) for N=8192 nodes, D=512 dims.

Sharding: columns of adj (= output rows) are split across the 8 cores;
x and U are replicated, so each core computes a [1024, 512] output slab
with no collectives.

Host-side restaging (pure layout shuffles, no arithmetic): every DRAM
tensor is laid out partition-major so each SBUF partition reads one long
contiguous run (16-32KB) — small per-row DMA packets were the original
bottleneck. The int32->bf16 and f32->bf16 casts ride the SWDGE DMA
engines for free.

Per-core kernel (all matmuls in bf16, accumulating in f32 PSUM):
  aggT[d, i] = sum_j x[j, d] * A[j, i]   via x-chunk weights, A streamed
  deg[i]     = sum_j A[j, i]             via an all-ones weight matrix
                                         (result replicated on 128 partitions)
  agg_scaled = aggT * (1/deg)  (free-dim broadcast multiply during PSUM evac)
  out[i, k]  = relu(sum_d agg_scaled[d, i] * U^T[d, k])
"""

import sys

if "/opt/trn_rl_repo" not in sys.path:
    sys.path.insert(0, "/opt/trn_rl_repo")

import numpy as np

import concourse.bacc as bacc
import concourse.mybir as mybir
import concourse.tile as tile
from concourse.bass_utils import run_bass_kernel_spmd

N = 8192          # nodes
D = 512           # node dim
NCORES = 8
SH = N // NCORES  # 1024 adj columns (output rows) per core
NJ = N // 128     # 64 contraction tiles
XG = 8            # j-tiles per load group
NG = NJ // XG     # 8 groups
F32 = mybir.dt.float32
BF16 = mybir.dt.bfloat16
I32 = mybir.dt.int32

_compiled = None


def _build():
    nc = bacc.Bacc("TRN2", target_bir_lowering=False, debug=False, num_devices=NCORES)
    # partition-major layouts (see _run for the host-side shuffles)
    x_d = nc.dram_tensor("x", [128, NJ, D], F32, kind="ExternalInput").ap()
    adj_d = nc.dram_tensor("adj", [2, 128, NJ, D], I32, kind="ExternalInput").ap()
    ut_d = nc.dram_tensor("ut", [128, 4, D], F32, kind="ExternalInput").ap()
    out_d = nc.dram_tensor("out", [128, 8, D], F32, kind="ExternalOutput").ap()

    with tile.TileContext(nc) as tc:
        with (
            tc.tile_pool(name="xw", bufs=1) as xw_pool,
            tc.tile_pool(name="abf", bufs=9) as abf_pool,
            tc.tile_pool(name="cons", bufs=1) as cons_pool,
            tc.tile_pool(name="evac", bufs=2) as evac_pool,
            tc.tile_pool(name="osb", bufs=2) as osb_pool,
            tc.tile_pool(name="pacc", bufs=1, space="PSUM") as pacc_pool,
            tc.tile_pool(name="pout", bufs=2, space="PSUM") as pout_pool,
        ):
            ones = cons_pool.tile([128, D], BF16)
            nc.vector.memset(ones[:], 1.0)
            # f32 identity for PE-transpose of the deg row
            ident = cons_pool.tile([128, 128], F32)
            nc.vector.memset(ident[:], 1.0)
            nc.gpsimd.affine_select(
                ident[:], ident[:], pattern=[[-1, 128]], base=0,
                channel_multiplier=1,
                compare_op=mybir.AluOpType.is_equal, fill=0.0,
            )
            u_bf = cons_pool.tile([128, 4, D], BF16)

            # dummy matmuls: PE filler issued where the h0 j-loop would
            # otherwise idle waiting on DMA, so the HAM clock gate never
            # sees an idle window and the PE stays at 2.4 GHz
            dummy_ps = pacc_pool.tile([128, D], F32, tag="deg", name="dummy")

            def pe_filler(n):
                for _ in range(n):
                    nc.tensor.matmul(
                        dummy_ps[:], ones[:, 0:128], ones[:],
                        start=True, stop=True, skip_group_check=True,
                    )

            xg_tiles = [None] * NG

            def load_x_group(g):
                xg = xw_pool.tile([128, XG, D], BF16, tag=f"xg{g}", name=f"xg{g}")
                nc.gpsimd.dma_start(xg[:], x_d[:, g * XG:(g + 1) * XG, :])
                xg_tiles[g] = xg

            for h in range(2):
                agg_ps = [
                    pacc_pool.tile([128, D], F32, tag=f"agg{c}", name=f"agg{c}")
                    for c in range(4)
                ]
                agg_sc = [
                    evac_pool.tile([128, D], BF16, tag=f"aggsc{c}", name=f"aggsc{c}")
                    for c in range(4)
                ]
                # per-partition partial degree counts; values stay <= NG so
                # bf16 accumulation is exact
                degp = evac_pool.tile([128, XG, D], BF16, tag="degp", bufs=2)
                nc.vector.memset(degp[:], 0.0)
                if h == 0:
                    pe_filler(28)
                for g in range(NG):
                    # interleave x-group loads with adj groups on the SWDGE
                    # queue so neither stream starves the other
                    if h == 0:
                        load_x_group(g)
                    a_bf = abf_pool.tile([128, XG, D], BF16, tag="abf")
                    nc.gpsimd.dma_start(
                        a_bf[:], adj_d[h, :, g * XG:(g + 1) * XG, :]
                    )
                    if h == 0 and g == 0:
                        nc.gpsimd.dma_start(u_bf[:], ut_d[:])
                    nc.vector.tensor_add(degp[:], degp[:], a_bf[:])
                    xg = xg_tiles[g]
                    for t in range(XG):
                        j = g * XG + t
                        st, sp = j == 0, j == NJ - 1
                        for c in range(4):
                            nc.tensor.matmul(
                                agg_ps[c][:],
                                xg[:, t, c * 128:(c + 1) * 128],
                                a_bf[:, t, :],
                                start=st,
                                stop=sp,
                            )
                            if sp:
                                # evacuate each chunk as soon as its
                                # accumulation closes (overlaps the
                                # remaining chunks' matmuls); on ACT so
                                # the DVE FIFO can never block stage 2
                                nc.scalar.copy(agg_sc[c][:], agg_ps[c][:])
                    if h == 0 and g < NG - 1:
                        pe_filler(12)

                # deg pipeline: sum the XG lanes and the partitions with
                # accumulating ones-matmuls (keeps the DVE off the
                # PE-critical path), then transpose into per-partition
                # layout for the output scale
                deg_ps = pacc_pool.tile([128, D], F32, tag="deg")
                for t in range(XG):
                    nc.tensor.matmul(
                        deg_ps[:], ones[:, 0:128], degp[:, t, :],
                        start=t == 0, stop=t == XG - 1,
                    )
                deg_sb = evac_pool.tile([128, D], F32, tag="degsb")
                nc.scalar.copy(deg_sb[:], deg_ps[:])
                degt_ps = pacc_pool.tile([128, 4, 128], F32, tag="deg")
                for ic in range(4):
                    nc.tensor.transpose(
                        degt_ps[:, ic, :],
                        deg_sb[:, ic * 128:(ic + 1) * 128],
                        ident[:],
                    )
                recipt = evac_pool.tile([128, 4], F32, tag="recipt")
                nc.vector.reciprocal_approx_fast(recipt[:], degt_ps[:, :, 0])

                out_sb = osb_pool.tile([128, 4, D], F32, tag="osb")
                for ic in range(4):
                    out_ps = pout_pool.tile([128, D], F32, tag="outps")
                    for c in range(4):
                        nc.tensor.matmul(
                            out_ps[:],
                            agg_sc[c][:, ic * 128:(ic + 1) * 128],
                            u_bf[:, c, :],
                            start=c == 0,
                            stop=c == 3,
                        )
                    # out = relu(out_raw / deg): positive scale commutes
                    # with relu, applied per partition in the activation
                    nc.scalar.activation(
                        out_sb[:, ic, :], out_ps[:],
                        mybir.ActivationFunctionType.Relu,
                        scale=recipt[:, ic:ic + 1],
                    )
                nc.sync.dma_start(out_d[:, h * 4:(h + 1) * 4, :], out_sb[:])

    nc.compile()
    return nc


def _get_compiled():
    global _compiled
    if _compiled is None:
        _compiled = _build()
    return _compiled


def _run(x, adj, u, **spmd_kwargs):
    nc = _get_compiled()
    x = np.asarray(x, dtype=np.float32)
    adj = np.asarray(adj, dtype=np.int32)
    u = np.asarray(u, dtype=np.float32)

    # x[t*128+p, d] -> x_r[p, t, d]
    x_r = np.ascontiguousarray(x.reshape(NJ, 128, D).transpose(1, 0, 2))
    # U^T[c*128+p, k] -> ut_r[p, c, k]
    ut_r = np.ascontiguousarray(u.T.reshape(4, 128, D).transpose(1, 0, 2))
    in_maps = []
    for core in range(NCORES):
        shard = adj[:, core * SH:(core + 1) * SH]
        # shard[t*128+p, h*512+d] -> adj_r[h, p, t, d]
        adj_r = np.ascontiguousarray(
            shard.reshape(NJ, 128, 2, D).transpose(2, 1, 0, 3)
        )
        in_maps.append({"x": x_r, "ut": ut_r, "adj": adj_r})

    res = run_bass_kernel_spmd(nc, in_maps, core_ids=list(range(NCORES)), **spmd_kwargs)
    # out_r[p, hic, k] -> out[hic*128+p, k], then stack core slabs
    out = np.concatenate(
        [
            res.results[c]["out"].transpose(1, 0, 2).reshape(SH, D)
            for c in range(NCORES)
        ],
        axis=0,
    )
    return out, res


def kernel(x, adj, U):
    out, _ = _run(x, adj, U)
    return out


# revision 27
# speedup vs baseline: 1.0643x; 1.0643x over previous
"""GNN message-passing layer on 8 TRN2 NeuronCores.

Computes out = relu((adj^T @ x / deg) @ U^T) for N=8192 nodes, D=512 dims.

Sharding: columns of adj (= output rows) are split across the 8 cores;
x and U are replicated, so each core computes a [1024, 512] output slab
with no collectives.

Host-side restaging (pure layout shuffles, no arithmetic): every DRAM
tensor is laid out partition-major so each SBUF partition reads one long
contiguous run (16-32KB) — small per-row DMA packets were the original
bottleneck. The int32->bf16 and f32->bf16 casts ride the SWDGE DMA
engines for free.

Per-core kernel (all matmuls in bf16, accumulating in f32 PSUM):
  aggT[d, i] = sum_j x[j, d] * A[j, i]   via x-chunk weights, A streamed
  deg[i]     = sum_j A[j, i]             via an all-ones weight matrix
                                         (result replicated on 128 partitions)
  agg_scaled = aggT * (1/deg)  (free-dim broadcast multiply during PSUM evac)
  out[i, k]  = relu(sum_d agg_scaled[d, i] * U^T[d, k])
"""

import sys

if "/opt/trn_rl_repo" not in sys.path:
    sys.path.insert(0, "/opt/trn_rl_repo")

import numpy as np

import concourse.bacc as bacc
import concourse.mybir as mybir
import concourse.tile as tile
from concourse.bass_utils import run_bass_kernel_spmd

N = 8192          # nodes
D = 512           # node dim
NCORES = 8
SH = N // NCORES  # 1024 adj columns (output rows) per core
NJ = N // 128     # 64 contraction tiles
XG = 8            # j-tiles per load group
NG = NJ // XG     # 8 groups
F32 = mybir.dt.float32
BF16 = mybir.dt.bfloat16
I32 = mybir.dt.int32

_compiled = None


def _build():
    nc = bacc.Bacc("TRN2", target_bir_lowering=False, debug=False, num_devices=NCORES)
    # partition-major layouts (see _run for the host-side shuffles)
    x_d = nc.dram_tensor("x", [128, NJ, D], F32, kind="ExternalInput").ap()
    adj_d = nc.dram_tensor("adj", [2, 128, NJ, D], I32, kind="ExternalInput").ap()
    ut_d = nc.dram_tensor("ut", [128, 4, D], F32, kind="ExternalInput").ap()
    out_d = nc.dram_tensor("out", [128, 8, D], F32, kind="ExternalOutput").ap()

    with tile.TileContext(nc) as tc:
        with (
            tc.tile_pool(name="xw", bufs=1) as xw_pool,
            tc.tile_pool(name="abf", bufs=6) as abf_pool,
            tc.tile_pool(name="cons", bufs=1) as cons_pool,
            tc.tile_pool(name="evac", bufs=2) as evac_pool,
            tc.tile_pool(name="osb", bufs=2) as osb_pool,
            tc.tile_pool(name="pacc", bufs=1, space="PSUM") as pacc_pool,
            tc.tile_pool(name="pout", bufs=2, space="PSUM") as pout_pool,
        ):
            ones = cons_pool.tile([128, D], BF16)
            nc.vector.memset(ones[:], 1.0)
            # f32 identity for PE-transpose of the deg row
            ident = cons_pool.tile([128, 128], F32)
            nc.vector.memset(ident[:], 1.0)
            nc.gpsimd.affine_select(
                ident[:], ident[:], pattern=[[-1, 128]], base=0,
                channel_multiplier=1,
                compare_op=mybir.AluOpType.is_equal, fill=0.0,
            )
            u_bf = cons_pool.tile([128, 4, D], BF16)

            # dummy matmuls: PE filler issued where the h0 j-loop would
            # otherwise idle waiting on DMA, so the HAM clock gate never
            # sees an idle window and the PE stays at 2.4 GHz
            dummy_ps = pacc_pool.tile([128, D], F32, tag="deg", name="dummy")

            def pe_filler(n):
                for _ in range(n):
                    nc.tensor.matmul(
                        dummy_ps[:], ones[:, 0:128], ones[:],
                        start=True, stop=True, skip_group_check=True,
                    )

            xg_tiles = [None] * NG

            def load_x_group(g):
                xg = xw_pool.tile([128, XG, D], BF16, tag=f"xg{g}", name=f"xg{g}")
                nc.gpsimd.dma_start(xg[:], x_d[:, g * XG:(g + 1) * XG, :])
                xg_tiles[g] = xg

            for h in range(2):
                agg_ps = [
                    pacc_pool.tile([128, D], F32, tag=f"agg{c}", name=f"agg{c}")
                    for c in range(4)
                ]
                agg_sc = [
                    evac_pool.tile([128, D], BF16, tag=f"aggsc{c}", name=f"aggsc{c}")
                    for c in range(4)
                ]
                # per-partition partial degree counts; values stay <= NJ so
                # bf16 accumulation is exact
                degp = evac_pool.tile([128, D], BF16, tag="degp", bufs=2)
                nc.vector.memset(degp[:], 0.0)
                if h == 0:
                    pe_filler(28)
                for g in range(NG):
                    # interleave x-group loads with adj groups on the SWDGE
                    # queue so neither stream starves the other
                    if h == 0:
                        load_x_group(g)
                    a_bf = abf_pool.tile([128, XG, D], BF16, tag="abf")
                    nc.gpsimd.dma_start(
                        a_bf[:], adj_d[h, :, g * XG:(g + 1) * XG, :]
                    )
                    if h == 0 and g == 0:
                        nc.gpsimd.dma_start(u_bf[:], ut_d[:])
                    xg = xg_tiles[g]
                    for t in range(XG):
                        j = g * XG + t
                        st, sp = j == 0, j == NJ - 1
                        nc.vector.tensor_add(degp[:], degp[:], a_bf[:, t, :])
                        for c in range(4):
                            nc.tensor.matmul(
                                agg_ps[c][:],
                                xg[:, t, c * 128:(c + 1) * 128],
                                a_bf[:, t, :],
                                start=st,
                                stop=sp,
                            )
                            if sp:
                                # evacuate each chunk as soon as its
                                # accumulation closes (overlaps the
                                # remaining chunks' matmuls); on ACT so
                                # the DVE FIFO can never block stage 2
                                nc.scalar.copy(agg_sc[c][:], agg_ps[c][:])
                    if h == 0 and g < NG - 1:
                        pe_filler(14)

                # deg pipeline: sum the XG lanes and the partitions with
                # accumulating ones-matmuls (keeps the DVE off the
                # PE-critical path), then transpose into per-partition
                # layout for the output scale
                deg_ps = pacc_pool.tile([128, D], F32, tag="deg")
                nc.tensor.matmul(
                    deg_ps[:], ones[:, 0:128], degp[:], start=True, stop=True
                )
                deg_sb = evac_pool.tile([128, D], F32, tag="degsb")
                nc.scalar.copy(deg_sb[:], deg_ps[:])
                degt_ps = pacc_pool.tile([128, 4, 128], F32, tag="deg")
                for ic in range(4):
                    nc.tensor.transpose(
                        degt_ps[:, ic, :],
                        deg_sb[:, ic * 128:(ic + 1) * 128],
                        ident[:],
                    )
                recipt = evac_pool.tile([128, 4], F32, tag="recipt")
                nc.vector.reciprocal_approx_fast(recipt[:], degt_ps[:, :, 0])

                out_sb = osb_pool.tile([128, 4, D], F32, tag="osb")
                for ic in range(4):
                    out_ps = pout_pool.tile([128, D], F32, tag="outps")
                    for c in range(4):
                        nc.tensor.matmul(
                            out_ps[:],
                            agg_sc[c][:, ic * 128:(ic + 1) * 128],
                            u_bf[:, c, :],
                            start=c == 0,
                            stop=c == 3,
                        )
                    # out = relu(out_raw / deg): positive scale commutes
                    # with relu, applied per partition in the activation
                    nc.scalar.activation(
                        out_sb[:, ic, :], out_ps[:],
                        mybir.ActivationFunctionType.Relu,
                        scale=recipt[:, ic:ic + 1],
                    )
                nc.sync.dma_start(out_d[:, h * 4:(h + 1) * 4, :], out_sb[:])

    nc.compile()
    return nc


def _get_compiled():
    global _compiled
    if _compiled is None:
        _compiled = _build()
    return _compiled


def _run(x, adj, u, **spmd_kwargs):
    nc = _get_compiled()
    x = np.asarray(x, dtype=np.float32)
    adj = np.asarray(adj, dtype=np.int32)
    u = np.asarray(u, dtype=np.float32)

    # x[t*128+p, d] -> x_r[p, t, d]
    x_r = np.ascontiguousarray(x.reshape(NJ, 128, D).transpose(1, 0, 2))
    # U^T[c*128+p, k] -> ut_r[p, c, k]
    ut_r = np.ascontiguousarray(u.T.reshape(4, 128, D).transpose(1, 0, 2))
    in_maps = []
    for core in range(NCORES):
        shard = adj[:, core * SH:(core + 1) * SH]
        # shard[t*128+p, h*512+d] -> adj_r[h, p, t, d]
        adj_r = np.ascontiguousarray(
            shard.reshape(NJ, 128, 2, D).transpose(2, 1, 0, 3)
        )
        in_maps.append({"x": x_r, "ut": ut_r, "adj": adj_r})

    res = run_bass_kernel_spmd(nc, in_maps, core_ids=list(range(NCORES)), **spmd_kwargs)
    # out_r[p, hic, k] -> out[hic*128+p, k], then stack core slabs
    out = np.concatenate(
        [
            res.results[c]["out"].transpose(1, 0, 2).reshape(SH, D)
            for c in range(NCORES)
        ],
        axis=0,
    )
    return out, res


def kernel(x, adj, U):
    out, _ = _run(x, adj, U)
    return out
